# revision 46
# baseline (speedup 1.0000x reference)
"""Trainium2 Bass kernel for nn_Net_76330158785143 (dense_cnn).

Pipeline per sample: per-sample 11x11 autocorrelation of channel 2 ->
conv5x5(1->32) relu -> maxpool2 -> conv5x5(32->64) relu -> maxpool2 ->
conv3x3(64->10) relu -> GAP -> log_softmax.

Sharding: pure data parallel, batch 8192 -> 1024 per core across 8 cores.

Layout notes (per 128-sample btile, 4 subs of 32 samples each):
- sample-in-sub index s = 8*g + c  (g: partition-group 0..3, c: chunk 0..7)
- corr: per-tap diag matmuls accumulating in PSUM, out columns split in two
  392-wide halves (PSUM bank cap), zero-padding rows/cols trimmed per tap
- conv1: 4 samples stacked on PE rows (K=4x25=100), out partition m=4*co+g
- conv2: dx 0..3 baked into dup_A row-groups (K=128, one matmul per dy);
  dx=4 taps as K=32 matmuls from dup2 (content pre-shifted by 4)
- conv3: all 9 taps accumulate into one PSUM region (K=64), two subs per
  N=256 stream
- schedule: corr(b+1) tap chunks interleaved into conv(b)'s sub-pipeline
  slots (depth-2 A/B software pipeline, s25g im2col prefetched 2 subs
  ahead and across the btile boundary via finish())
"""

import sys

sys.path.insert(0, "/opt/trn_rl_repo")

import numpy as np

import concourse.bacc as bacc
import concourse.mybir as mybir
from concourse.ap import AP
from concourse.tile import TileContext
from concourse.bass_utils import run_bass_kernel_spmd

F32 = mybir.dt.float32
BF16 = mybir.dt.bfloat16
ALU = mybir.AluOpType
ACTF = mybir.ActivationFunctionType
AXIS = mybir.AxisListType

N_CORES = 8
B_FULL = 8192
B_CORE = B_FULL // N_CORES


def _build(nc, b_core):
    n_bt = b_core // 128

    x_d = nc.dram_tensor("x", [b_core, 3, 28, 28], F32, kind="ExternalInput")
    identp_d = nc.dram_tensor("identp", [128, 128], BF16, kind="ExternalInput")
    ident10p_d = nc.dram_tensor("ident10p", [16, 16], F32, kind="ExternalInput")
    w1x4_d = nc.dram_tensor("w1x4", [100, 128], BF16, kind="ExternalInput")
    b1x4_d = nc.dram_tensor("b1x4", [128, 1], F32, kind="ExternalInput")
    w2dx_d = nc.dram_tensor("w2dx", [128, 320], BF16, kind="ExternalInput")
    w2x4_d = nc.dram_tensor("w2x4", [32, 320], BF16, kind="ExternalInput")
    b2p_d = nc.dram_tensor("b2p", [64, 1], F32, kind="ExternalInput")
    w3n_d = nc.dram_tensor("w3n", [64, 96], BF16, kind="ExternalInput")
    b3q_d = nc.dram_tensor("b3q", [16, 1], F32, kind="ExternalInput")
    out_d = nc.dram_tensor("out", [b_core, 10], F32, kind="ExternalOutput")

    with TileContext(nc) as tc:
        cpool_cm = tc.tile_pool(name="const", bufs=1)
        cpool = cpool_cm.__enter__()

        def _load_const(name, dram, shape, dtype):
            t = cpool.tile(shape, dtype, name=name + "_sb")
            f = int(np.prod(shape[1:]))
            nc.sync.dma_start(
                out=AP(t.tensor, 0, [[f, shape[0]], [1, f]]),
                in_=AP(dram, 0, [[f, shape[0]], [1, f]]),
            )
            return t

        ident = _load_const("ident", identp_d, [128, 128], BF16)
        ident10 = _load_const("ident10", ident10p_d, [16, 16], F32)
        w1x4_sb = _load_const("w1x4", w1x4_d, [100, 128], BF16)
        b1x4_sb = _load_const("b1x4", b1x4_d, [128, 1], F32)
        w2dx_sb = _load_const("w2dx", w2dx_d, [128, 320], BF16)
        w2x4_sb = _load_const("w2x4", w2x4_d, [32, 320], BF16)
        b2p_sb = _load_const("b2p", b2p_d, [64, 1], F32)
        w3n_sb = _load_const("w3n", w3n_d, [64, 96], BF16)
        b3q_sb = _load_const("b3q", b3q_d, [16, 1], F32)
        # zero pad rows for corr_d tail (s25g shifted reads run past row 128)
        zpad = cpool.tile([8, 924], BF16, name="zpad_sb")
        nc.vector.memset(zpad[:, :], 0.0)

        from contextlib import ExitStack

        with ExitStack() as stack:
            ent = stack.enter_context
            imgpool = ent(tc.tile_pool(name="img", bufs=2))
            tmplpool = ent(tc.tile_pool(name="tmpl", bufs=2))
            diagpool = ent(tc.tile_pool(name="diag", bufs=8))
            corrpool = ent(tc.tile_pool(name="corr", bufs=2))
            s25pool = ent(tc.tile_pool(name="s25", bufs=6))
            pxpool = ent(tc.tile_pool(name="px", bufs=3))
            pyrpool = ent(tc.tile_pool(name="pyr", bufs=2))
            pypool = ent(tc.tile_pool(name="py", bufs=3))
            dupApool = ent(tc.tile_pool(name="dupA", bufs=3))
            dup2pool = ent(tc.tile_pool(name="dup2", bufs=3))
            o2pool = ent(tc.tile_pool(name="o2", bufs=4))
            qxpool = ent(tc.tile_pool(name="qx", bufs=4))
            l3pool = ent(tc.tile_pool(name="l3", bufs=2))
            smpool = ent(tc.tile_pool(name="sm", bufs=4))
            lgbpool = ent(tc.tile_pool(name="lgb", bufs=2))
            dscrpool = ent(tc.tile_pool(name="dscr", bufs=2, space="DRAM"))
            pcorr = ent(tc.tile_pool(name="pcorr", bufs=1, space="PSUM"))
            pc1 = ent(tc.tile_pool(name="pc1", bufs=3, space="PSUM"))
            pc2 = ent(tc.tile_pool(name="pc2", bufs=2, space="PSUM"))
            pc3 = ent(tc.tile_pool(name="pc3", bufs=1, space="PSUM"))
            pools = dict(
                imgpool=imgpool, tmplpool=tmplpool, diagpool=diagpool,
                corrpool=corrpool, s25pool=s25pool,
                pxpool=pxpool, pyrpool=pyrpool, pypool=pypool,
                dupApool=dupApool, dup2pool=dup2pool, o2pool=o2pool,
                qxpool=qxpool, l3pool=l3pool, smpool=smpool,
                lgbpool=lgbpool, dscrpool=dscrpool, pcorr=pcorr,
                pc1=pc1, pc2=pc2, pc3=pc3,
            )
            consts = dict(
                ident=ident, ident10=ident10, w1x4_sb=w1x4_sb,
                b1x4_sb=b1x4_sb, w2dx_sb=w2dx_sb, w2x4_sb=w2x4_sb,
                b2p_sb=b2p_sb, w3n_sb=w3n_sb,
                b3q_sb=b3q_sb, zpad=zpad,
            )
            # software pipeline: corr(b) tap chunks are interleaved into
            # conv(b-1)'s sub-pipeline slots so every dup-DMA latency
            # window has PE work to chew on.
            state = {}
            ctxs = {}
            pre = None
            for b in range(n_bt + 1):
                if b < n_bt:
                    ctxs[b] = _corr_begin(nc, b, x_d, pools, consts)
                if b >= 1:
                    cx = ctxs.get(b)
                    # btile 1 has no cross-boundary s25g prefetch; put some
                    # of corr(1)'s taps in front of conv(0) to cover the
                    # corr_d(0) -> s25g(0) handoff latency
                    ts = 0
                    if b == 1 and cx is not None:
                        _corr_taps(nc, cx, 0, 25, pools, consts)
                        ts = 25

                    def filler(k, nslots, cx=cx, ts=ts):
                        if cx is None:
                            return
                        t0 = ts + ((121 - ts) * k) // nslots
                        t1 = ts + ((121 - ts) * (k + 1)) // nslots
                        if t1 > t0:
                            _corr_taps(nc, cx, t0, t1, pools, consts)

                    def finish(b=b, cx=cx):
                        if cx is None:
                            return None
                        cd = _corr_end(nc, b, cx, pools, consts)
                        state[b] = cd
                        ctxs.pop(b)
                        return {
                            0: _s25g_fetch(nc, 0, cd, pools),
                            1: _s25g_fetch(nc, 1, cd, pools),
                        }

                    pre = _conv_stage(
                        nc, b - 1, state.pop(b - 1), out_d, pools, consts,
                        filler, finish, pre,
                    )
                else:
                    _corr_taps(nc, ctxs[0], 0, 121, pools, consts)
                    state[0] = _corr_end(nc, 0, ctxs.pop(0), pools, consts)

        cpool_cm.__exit__(None, None, None)
    return nc


def _corr_begin(nc, b, x_d, P, C):
    imgpool = P["imgpool"]; tmplpool = P["tmplpool"]; pcorr = P["pcorr"]
    # channel 2 into zero-padded 38x38, cast bf16
    img = imgpool.tile([128, 38 * 38], BF16)
    nc.gpsimd.memset(img[:, :], 0.0)
    nc.gpsimd.dma_start(
        out=AP(img.tensor, 5 * 38 + 5, [[1444, 128], [38, 28], [1, 28]]),
        in_=AP(x_d, b * 128 * 2352 + 2 * 784, [[2352, 128], [1, 784]]),
    )
    # template = center 11x11 crop
    tmpl = tmplpool.tile([128, 128], F32)
    nc.vector.tensor_copy(
        out=AP(tmpl.tensor, 0, [[128, 128], [1, 121]]),
        in_=AP(img.tensor, 13 * 38 + 13, [[1444, 128], [38, 11], [1, 11]]),
    )
    ps_a = pcorr.tile([128, 392], F32, tag="corr_a")
    ps_b = pcorr.tile([128, 392], F32, tag="corr_b")
    return dict(img=img, tmpl=tmpl, ps_a=ps_a, ps_b=ps_b)


def _corr_taps(nc, cx, t0, t1, P, C):
    """Accumulating diag matmuls for taps [t0, t1)."""
    img, tmpl = cx["img"], cx["tmpl"]
    ps_a, ps_b = cx["ps_a"], cx["ps_b"]
    for t in range(t0, t1):
        u, v = t // 11, t % 11
        dg = P["diagpool"].tile([128, 128], BF16)
        nc.vector.tensor_scalar_mul(dg[:, :], C["ident"][:, :], tmpl[:, t : t + 1])
        if t == 0:
            # full extent (zero-padded rows contribute 0) to reset PSUM
            y0, y1, x0, x1 = 0, 28, 0, 28
        else:
            # rows/cols whose img window is entirely zero padding are skipped
            y0, y1 = max(0, 5 - u), min(28, 33 - u)
            x0, x1 = max(0, 5 - v), min(28, 33 - v)
        ya0, ya1 = y0, min(14, y1)
        yb0, yb1 = max(14, y0), y1
        nc.tensor.matmul(
            AP(ps_a.tensor, ya0 * 28 + x0, [[392, 128], [28, ya1 - ya0], [1, x1 - x0]]),
            dg[:, :],
            AP(
                img.tensor,
                (ya0 + u) * 38 + (x0 + v),
                [[1444, 128], [38, ya1 - ya0], [1, x1 - x0]],
            ),
            start=(t == 0), stop=(t == 120), skip_group_check=True,
        )
        nc.tensor.matmul(
            AP(ps_b.tensor, (yb0 - 14) * 28 + x0, [[392, 128], [28, yb1 - yb0], [1, x1 - x0]]),
            dg[:, :],
            AP(
                img.tensor,
                (yb0 + u) * 38 + (x0 + v),
                [[1444, 128], [38, yb1 - yb0], [1, x1 - x0]],
            ),
            start=(t == 0), stop=(t == 120), skip_group_check=True,
        )


def _corr_end(nc, b, cx, P, C):
    corr = P["corrpool"].tile([128, 924], BF16)
    nc.vector.tensor_copy(out=corr[:, 0:392], in_=cx["ps_a"][:, :])
    nc.vector.tensor_copy(out=corr[:, 392:784], in_=cx["ps_b"][:, :])
    nc.gpsimd.memset(corr[:, 784:924], 0.0)
    corr_d = P["dscrpool"].tile([136, 924], BF16, tag="corr_d")
    nc.sync.dma_start(
        out=AP(corr_d.tensor, 0, [[924, 128], [1, 924]]),
        in_=corr[:, :],
    )
    # zero tail rows (s25g shifted reads overrun into them)
    nc.sync.dma_start(
        out=AP(corr_d.tensor, 128 * 924, [[924, 8], [1, 924]]),
        in_=C["zpad"][:, :],
    )
    return corr_d


def _conv_stage(nc, b, corr_d, out_d, P, C, filler=None, finish=None, pre=None):
    logitsb = P["lgbpool"].tile([16, 128], F32)
    # sub-level software pipeline, depth 2: A(s+2) is issued before B(s)
    # so PE always has conv1 work while dup DMAs for the next B transfer.
    # filler(k, 5) pads each B stage with the NEXT btile's corr taps;
    # finish() flushes that corr to DRAM during the tail B stages so the
    # next btile's first s25g prefetches cross the boundary early.
    dups = {}
    l3s = {}
    if pre is None:
        s25 = {0: _s25g_fetch(nc, 0, corr_d, P), 1: _s25g_fetch(nc, 1, corr_d, P)}
    else:
        s25 = pre
    nxt = None
    for s in range(6):
        if s + 2 < 4:
            s25[s + 2] = _s25g_fetch(nc, s + 2, corr_d, P)
        if s < 4:
            dups[s] = _conv_a(nc, b, s, s25.pop(s), P, C)
        if filler is not None and s < 5:
            filler(s, 5)
        if s == 5 and finish is not None:
            nxt = finish()
        if s >= 2:
            _conv_b(nc, b, s - 2, dups.pop(s - 2), logitsb, l3s, P, C)
    _softmax_out(nc, b, logitsb, out_d, P, C)
    return nxt


def _s25g_fetch(nc, sub, corr_d, P):
    """im2col prefetch: s25g[20*dy+4*dx+g, c*924+j] =
    corr[32*sub+8*g+c, dy*28+dx + j]"""
    s25g = P["s25pool"].tile([100, 7392], BF16)
    for dy in range(5):
        nc.gpsimd.dma_start(
            out=s25g[20 * dy : 20 * dy + 20, :],
            in_=AP(
                corr_d.tensor,
                sub * 32 * 924 + dy * 28,
                [[1, 5], [7392, 4], [1, 7392]],
            ),
        )
    return s25g


def _conv_a(nc, b, sub, s25g, P, C):
    """conv1 -> maxpool (raw) -> bias+relu -> shifted dups."""
    w1x4_sb = C["w1x4_sb"]; b1x4_sb = C["b1x4_sb"]
    px_all = P["pxpool"].tile([128, 2304], BF16)
    for c in range(8):
        for h in range(2):
            ps1 = P["pc1"].tile([128, 288], F32, tag="ps1")
            nc.tensor.matmul(
                ps1[:, :],
                w1x4_sb[0:100, :],
                AP(
                    s25g.tensor,
                    c * 924 + h * 336,
                    [[7392, 100], [28, 12], [1, 24]],
                ),
                start=True, stop=True,
            )
            # maxpool x-pairs straight off PSUM (bias/relu commute with max)
            nc.vector.tensor_reduce(
                out=px_all[:, c * 288 + h * 144 : c * 288 + h * 144 + 144],
                in_=AP(ps1.tensor, 0, [[288, 128], [24, 12], [2, 12], [1, 2]]),
                axis=AXIS.X,
                op=ALU.max,
            )
    # maxpool y-pairs then one bias+relu -> pooled [128=(4co+g), (c,12,12)]
    py_raw = P["pyrpool"].tile([128, 1152], BF16)
    nc.vector.tensor_max(
        py_raw[:, :],
        AP(px_all.tensor, 0, [[2304, 128], [24, 96], [1, 12]]),
        AP(px_all.tensor, 12, [[2304, 128], [24, 96], [1, 12]]),
    )
    py_all = P["pypool"].tile([128, 1204], BF16)
    nc.scalar.activation(
        py_all[:, 0:1152], py_raw[:, :], ACTF.Relu, bias=b1x4_sb[:, 0:1]
    )
    nc.gpsimd.memset(py_all[:, 1152:1204], 0.0)
    # shifted dups (partition remap to [32ci, ...]):
    # dup_A row-group r bakes dx=r (shift r elems); dup2 bakes dx=4
    dup_A = P["dupApool"].tile([128, 4612], BF16)
    dup2 = P["dup2pool"].tile([32, 4608], BF16)
    for r in range(4):
        nc.sync.dma_start(
            out=AP(
                dup_A.tensor,
                32 * r * 4612 + (4 - r),
                [[4612, 32], [1152, 4], [1, 1152]],
            ),
            in_=AP(py_all.tensor, 0, [[1204, 128], [1, 1152]]),
        )
    nc.sync.dma_start(
        out=AP(dup2.tensor, 0, [[4608, 32], [1152, 4], [1, 1152]]),
        in_=AP(py_all.tensor, 4, [[1204, 128], [1, 1152]]),
    )
    return dup_A, dup2


def _conv_b(nc, b, sub, dup_pair, logitsb, l3s, P, C):
    """conv2 (dy via rhs offset, dx via dup row-groups) -> pool -> conv3 -> GAP."""
    dup_A, dup2 = dup_pair
    w2dx_sb = C["w2dx_sb"]; w2x4_sb = C["w2x4_sb"]; b2p_sb = C["b2p_sb"]
    w3n_sb = C["w3n_sb"]; b3q_sb = C["b3q_sb"]

    if sub % 2 == 0:
        l3s["t"] = P["l3pool"].tile([64, 1024], BF16, name="l3pair")
    l3 = l3s["t"]
    half = (sub % 2) * 512
    for cc in range(4):
        ps2 = P["pc2"].tile([64, 512], F32, tag="ps2")
        for dy in range(5):
            nc.tensor.matmul(
                ps2[:, :],
                w2dx_sb[:, 64 * dy : 64 * dy + 64],
                AP(
                    dup_A.tensor,
                    4 + cc * 1152 + dy * 12,
                    [[4612, 128], [144, 8], [12, 8], [1, 8]],
                ),
                start=(dy == 0), stop=False,
            )
        # taps (dy, dx=4): K=32 via dup2 (content pre-shifted by 4)
        for dy in range(5):
            nc.tensor.matmul(
                ps2[:, :],
                w2x4_sb[:, 64 * dy : 64 * dy + 64],
                AP(
                    dup2.tensor,
                    cc * 1152 + dy * 12,
                    [[4608, 32], [144, 8], [12, 8], [1, 8]],
                ),
                start=False, stop=(dy == 4),
                tile_position=(0, 0),
            )
        # bias+relu off PSUM on ACT, then maxpool 2x2 on DVE into l3
        o2 = P["o2pool"].tile([64, 512], BF16)
        nc.scalar.activation(o2[:, :], ps2[:, :], ACTF.Relu, bias=b2p_sb[:, 0:1])
        qx = P["qxpool"].tile([64, 256], BF16)
        nc.vector.tensor_reduce(
            out=qx[:, :],
            in_=AP(o2.tensor, 0, [[512, 64], [8, 64], [2, 4], [1, 2]]),
            axis=AXIS.X,
            op=ALU.max,
        )
        nc.vector.tensor_max(
            l3[:, half + cc * 128 : half + cc * 128 + 128],
            AP(qx.tensor, 0, [[256, 64], [32, 8], [8, 4], [1, 4]]),
            AP(qx.tensor, 4, [[256, 64], [32, 8], [8, 4], [1, 4]]),
        )

    if sub % 2 == 0:
        return
    # bias+relu once for the pair -> conv3 input [64ci, (64s,16pix)]
    # conv3: 9 taps, K=64, one PSUM region, both subs in one N=256 stream
    ps3 = P["pc3"].tile([16, 256], F32, tag="ps3")
    for t in range(9):
        dy, dx = t // 3, t % 3
        nc.tensor.matmul(
            ps3[0:10, :],
            w3n_sb[0:64, 10 * t : 10 * t + 10],
            AP(l3.tensor, dy * 4 + dx, [[1024, 64], [16, 64], [4, 2], [1, 2]]),
            start=(t == 0), stop=(t == 8),
        )
    # relu(0.25*x + 0.25*b3) then sum 4 pix = GAP of relu(x+b3)
    ga = P["smpool"].tile([16, 256], F32, tag="ga")
    nc.scalar.activation(
        ga[0:10, :], ps3[0:10, :], ACTF.Relu, bias=b3q_sb[0:10, 0:1], scale=0.25
    )
    nc.vector.tensor_reduce(
        out=logitsb[0:10, (sub - 1) * 32 : (sub + 1) * 32],
        in_=AP(ga.tensor, 0, [[256, 10], [4, 64], [1, 4]]),
        axis=AXIS.X,
        op=ALU.add,
    )


def _softmax_out(nc, b, logitsb, out_d, P, C):
    smpool = P["smpool"]
    psT = P["pc1"].tile([128, 16], F32, tag="ps1")
    nc.tensor.transpose(psT[:, 0:10], logitsb[0:10, :], C["ident10"][0:10, 0:10])
    mx = smpool.tile([128, 1], F32, tag="mx")
    nc.vector.reduce_max(mx[:, :], psT[:, 0:10], axis=AXIS.X)
    hs = smpool.tile([128, 16], F32, tag="hs")
    nc.vector.tensor_scalar(hs[:, 0:10], psT[:, 0:10], mx[:, 0:1], None, ALU.subtract)
    ex = smpool.tile([128, 16], F32, tag="ex")
    nc.scalar.activation(ex[:, 0:10], hs[:, 0:10], ACTF.Exp)
    sm = smpool.tile([128, 1], F32, tag="sm")
    nc.vector.reduce_sum(sm[:, :], ex[:, 0:10], axis=AXIS.X)
    lsm = smpool.tile([128, 1], F32, tag="lsm")
    nc.scalar.activation(lsm[:, :], sm[:, :], ACTF.Ln)
    outt = smpool.tile([128, 16], F32, tag="outt")
    nc.vector.tensor_scalar(outt[:, 0:10], hs[:, 0:10], lsm[:, 0:1], None, ALU.subtract)
    nc.sync.dma_start(
        out=AP(out_d, b * 1280, [[10, 128], [1, 10]]),
        in_=outt[:, 0:10],
    )


_CACHE = {}


def _get_nc(b_core):
    if b_core not in _CACHE:
        nc = bacc.Bacc(
            "TRN2",
            target_bir_lowering=False,
            debug=False,
            num_devices=N_CORES,
            num_swdge_queues=2,
        )
        _build(nc, b_core)
        nc.compile()
        _CACHE[b_core] = nc
    return _CACHE[b_core]


def _prep_inputs(inputs):
    import ml_dtypes

    bf16 = ml_dtypes.bfloat16
    w1 = np.asarray(inputs["w1"], dtype=np.float32).reshape(32, 25)
    w2 = np.asarray(inputs["w2"], dtype=np.float32).reshape(64, 32, 5, 5)
    w3 = np.asarray(inputs["w3"], dtype=np.float32).reshape(10, 64, 9)
    b1 = np.asarray(inputs["b1"], dtype=np.float32)
    b2 = np.asarray(inputs["b2"], dtype=np.float32)
    b3 = np.asarray(inputs["b3"], dtype=np.float32)

    # conv1: w1x4[4*t+g, 4*co+g] = w1[co, t]
    w1x4 = np.zeros((100, 128), dtype=np.float32)
    for t in range(25):
        for g in range(4):
            w1x4[4 * t + g, 4 * np.arange(32) + g] = w1[:, t]
    b1x4 = np.zeros((128, 1), dtype=np.float32)
    for co in range(32):
        for g in range(4):
            b1x4[4 * co + g, 0] = b1[co]

    # conv2: w2dx[32*r+ci, 64*dy+co] = w2[co, ci, dy, r] (r=dx 0..3)
    w2dx = np.zeros((128, 320), dtype=np.float32)
    for r in range(4):
        for dy in range(5):
            w2dx[32 * r : 32 * r + 32, 64 * dy : 64 * dy + 64] = w2[:, :, dy, r].T
    # w2x4[ci, 64*dy+co] = w2[co, ci, dy, 4]
    w2x4 = np.zeros((32, 320), dtype=np.float32)
    for dy in range(5):
        w2x4[:, 64 * dy : 64 * dy + 64] = w2[:, :, dy, 4].T

    # conv3: w3n[ci, 10*t+co] = w3[co, ci, t]
    w3n = np.zeros((64, 96), dtype=np.float32)
    for t in range(9):
        w3n[:, 10 * t : 10 * t + 10] = w3[:, :, t].T
    b3q = np.zeros((16, 1), dtype=np.float32)
    b3q[0:10, 0] = 0.25 * b3

    return dict(
        identp=np.eye(128, dtype=bf16),
        ident10p=np.eye(16, dtype=np.float32),
        w1x4=w1x4.astype(bf16),
        b1x4=b1x4,
        w2dx=w2dx.astype(bf16),
        w2x4=w2x4.astype(bf16),
        b2p=b2.reshape(64, 1),
        w3n=w3n.astype(bf16),
        b3q=b3q,
    )


def _run(inputs, b_core=B_CORE, trace=False):
    x = np.ascontiguousarray(np.asarray(inputs["x"], dtype=np.float32))
    consts = _prep_inputs(inputs)
    nc = _get_nc(b_core)
    in_maps = [
        {"x": x[i * b_core : (i + 1) * b_core], **consts} for i in range(N_CORES)
    ]
    res = run_bass_kernel_spmd(nc, in_maps, core_ids=list(range(N_CORES)), trace=trace)
    out = np.concatenate([res.results[i]["out"] for i in range(N_CORES)], axis=0)
    return out.astype(np.float32), res


def kernel(**inputs) -> np.ndarray:
    out, _ = _run(inputs)
    return out


# revision 47
# speedup vs baseline: 1.0117x; 1.0117x over previous
"""Trainium2 Bass kernel for nn_Net_76330158785143 (dense_cnn).

Pipeline per sample: per-sample 11x11 autocorrelation of channel 2 ->
conv5x5(1->32) relu -> maxpool2 -> conv5x5(32->64) relu -> maxpool2 ->
conv3x3(64->10) relu -> GAP -> log_softmax.

Sharding: pure data parallel, batch 8192 -> 1024 per core across 8 cores.

Layout notes (per 128-sample btile, 4 subs of 32 samples each):
- sample-in-sub index s = 8*g + c  (g: partition-group 0..3, c: chunk 0..7)
- corr: per-tap diag matmuls accumulating in PSUM, out columns split in two
  392-wide halves (PSUM bank cap), zero-padding rows/cols trimmed per tap
- conv1: 4 samples stacked on PE rows (K=4x25=100), out partition m=4*co+g
- conv2: dx 0..3 baked into dup_A row-groups (K=128, one matmul per dy);
  dx=4 taps as K=32 matmuls from dup2 (content pre-shifted by 4)
- conv3: all 9 taps accumulate into one PSUM region (K=64), two subs per
  N=256 stream
- schedule: corr(b+1) tap chunks interleaved into conv(b)'s sub-pipeline
  slots (depth-2 A/B software pipeline, s25g im2col prefetched 2 subs
  ahead and across the btile boundary via finish())
"""

import sys

sys.path.insert(0, "/opt/trn_rl_repo")

import numpy as np

import concourse.bacc as bacc
import concourse.mybir as mybir
from concourse.ap import AP
from concourse.tile import TileContext
from concourse.bass_utils import run_bass_kernel_spmd

F32 = mybir.dt.float32
BF16 = mybir.dt.bfloat16
ALU = mybir.AluOpType
ACTF = mybir.ActivationFunctionType
AXIS = mybir.AxisListType

N_CORES = 8
B_FULL = 8192
B_CORE = B_FULL // N_CORES


def _build(nc, b_core):
    n_bt = b_core // 128

    x_d = nc.dram_tensor("x", [b_core, 3, 28, 28], F32, kind="ExternalInput")
    identp_d = nc.dram_tensor("identp", [128, 128], BF16, kind="ExternalInput")
    ident10p_d = nc.dram_tensor("ident10p", [16, 16], F32, kind="ExternalInput")
    w1x4_d = nc.dram_tensor("w1x4", [100, 128], BF16, kind="ExternalInput")
    b1x4_d = nc.dram_tensor("b1x4", [128, 1], F32, kind="ExternalInput")
    w2dx_d = nc.dram_tensor("w2dx", [128, 320], BF16, kind="ExternalInput")
    w2x4_d = nc.dram_tensor("w2x4", [32, 320], BF16, kind="ExternalInput")
    b2p_d = nc.dram_tensor("b2p", [64, 1], F32, kind="ExternalInput")
    w3n_d = nc.dram_tensor("w3n", [64, 96], BF16, kind="ExternalInput")
    b3q_d = nc.dram_tensor("b3q", [16, 1], F32, kind="ExternalInput")
    out_d = nc.dram_tensor("out", [b_core, 10], F32, kind="ExternalOutput")

    with TileContext(nc) as tc:
        cpool_cm = tc.tile_pool(name="const", bufs=1)
        cpool = cpool_cm.__enter__()

        def _load_const(name, dram, shape, dtype):
            t = cpool.tile(shape, dtype, name=name + "_sb")
            f = int(np.prod(shape[1:]))
            nc.sync.dma_start(
                out=AP(t.tensor, 0, [[f, shape[0]], [1, f]]),
                in_=AP(dram, 0, [[f, shape[0]], [1, f]]),
            )
            return t

        ident = _load_const("ident", identp_d, [128, 128], BF16)
        ident10 = _load_const("ident10", ident10p_d, [16, 16], F32)
        w1x4_sb = _load_const("w1x4", w1x4_d, [100, 128], BF16)
        b1x4_sb = _load_const("b1x4", b1x4_d, [128, 1], F32)
        w2dx_sb = _load_const("w2dx", w2dx_d, [128, 320], BF16)
        w2x4_sb = _load_const("w2x4", w2x4_d, [32, 320], BF16)
        b2p_sb = _load_const("b2p", b2p_d, [64, 1], F32)
        w3n_sb = _load_const("w3n", w3n_d, [64, 96], BF16)
        b3q_sb = _load_const("b3q", b3q_d, [16, 1], F32)
        # zero pad rows for corr_d tail (s25g shifted reads run past row 128)
        zpad = cpool.tile([8, 924], BF16, name="zpad_sb")
        nc.vector.memset(zpad[:, :], 0.0)

        from contextlib import ExitStack

        with ExitStack() as stack:
            ent = stack.enter_context
            imgpool = ent(tc.tile_pool(name="img", bufs=2))
            tmplpool = ent(tc.tile_pool(name="tmpl", bufs=2))
            diagpool = ent(tc.tile_pool(name="diag", bufs=8))
            corrpool = ent(tc.tile_pool(name="corr", bufs=2))
            s25pool = ent(tc.tile_pool(name="s25", bufs=6))
            pxpool = ent(tc.tile_pool(name="px", bufs=3))
            pyrpool = ent(tc.tile_pool(name="pyr", bufs=2))
            pypool = ent(tc.tile_pool(name="py", bufs=3))
            dupApool = ent(tc.tile_pool(name="dupA", bufs=3))
            dup2pool = ent(tc.tile_pool(name="dup2", bufs=3))
            o2pool = ent(tc.tile_pool(name="o2", bufs=4))
            qxpool = ent(tc.tile_pool(name="qx", bufs=4))
            l3pool = ent(tc.tile_pool(name="l3", bufs=2))
            smpool = ent(tc.tile_pool(name="sm", bufs=4))
            lgbpool = ent(tc.tile_pool(name="lgb", bufs=2))
            dscrpool = ent(tc.tile_pool(name="dscr", bufs=2, space="DRAM"))
            pcorr = ent(tc.tile_pool(name="pcorr", bufs=1, space="PSUM"))
            pc1 = ent(tc.tile_pool(name="pc1", bufs=3, space="PSUM"))
            pc2 = ent(tc.tile_pool(name="pc2", bufs=2, space="PSUM"))
            pc3 = ent(tc.tile_pool(name="pc3", bufs=1, space="PSUM"))
            pools = dict(
                imgpool=imgpool, tmplpool=tmplpool, diagpool=diagpool,
                corrpool=corrpool, s25pool=s25pool,
                pxpool=pxpool, pyrpool=pyrpool, pypool=pypool,
                dupApool=dupApool, dup2pool=dup2pool, o2pool=o2pool,
                qxpool=qxpool, l3pool=l3pool, smpool=smpool,
                lgbpool=lgbpool, dscrpool=dscrpool, pcorr=pcorr,
                pc1=pc1, pc2=pc2, pc3=pc3,
            )
            consts = dict(
                ident=ident, ident10=ident10, w1x4_sb=w1x4_sb,
                b1x4_sb=b1x4_sb, w2dx_sb=w2dx_sb, w2x4_sb=w2x4_sb,
                b2p_sb=b2p_sb, w3n_sb=w3n_sb,
                b3q_sb=b3q_sb, zpad=zpad,
            )
            # software pipeline: corr(b) tap chunks are interleaved into
            # conv(b-1)'s sub-pipeline slots so every dup-DMA latency
            # window has PE work to chew on.
            state = {}
            ctxs = {}
            pre = None
            for b in range(n_bt + 1):
                if b < n_bt:
                    ctxs[b] = _corr_begin(nc, b, x_d, pools, consts)
                if b >= 1:
                    cx = ctxs.get(b)
                    # btile 1 has no cross-boundary s25g prefetch; put some
                    # of corr(1)'s taps in front of conv(0) to cover the
                    # corr_d(0) -> s25g(0) handoff latency
                    ts = 0
                    if b == 1 and cx is not None:
                        _corr_taps(nc, cx, 0, 25, pools, consts)
                        ts = 25

                    def filler(k, nslots, cx=cx, ts=ts):
                        if cx is None:
                            return
                        t0 = ts + ((121 - ts) * k) // nslots
                        t1 = ts + ((121 - ts) * (k + 1)) // nslots
                        if t1 > t0:
                            _corr_taps(nc, cx, t0, t1, pools, consts)

                    def finish(b=b, cx=cx):
                        if cx is None:
                            return None
                        cd = _corr_end(nc, b, cx, pools, consts)
                        state[b] = cd
                        ctxs.pop(b)
                        return {
                            0: _s25g_fetch(nc, 0, cd, pools),
                            1: _s25g_fetch(nc, 1, cd, pools),
                        }

                    pre = _conv_stage(
                        nc, b - 1, state.pop(b - 1), out_d, pools, consts,
                        filler, finish, pre,
                    )
                else:
                    _corr_taps(nc, ctxs[0], 0, 121, pools, consts)
                    state[0] = _corr_end(nc, 0, ctxs.pop(0), pools, consts)

        cpool_cm.__exit__(None, None, None)
    return nc


def _corr_begin(nc, b, x_d, P, C):
    imgpool = P["imgpool"]; tmplpool = P["tmplpool"]; pcorr = P["pcorr"]
    # channel 2 into zero-padded 38x38, cast bf16
    img = imgpool.tile([128, 38 * 38], BF16)
    nc.gpsimd.memset(img[:, :], 0.0)
    nc.gpsimd.dma_start(
        out=AP(img.tensor, 5 * 38 + 5, [[1444, 128], [38, 28], [1, 28]]),
        in_=AP(x_d, b * 128 * 2352 + 2 * 784, [[2352, 128], [1, 784]]),
    )
    # template = center 11x11 crop
    tmpl = tmplpool.tile([128, 128], F32)
    nc.vector.tensor_copy(
        out=AP(tmpl.tensor, 0, [[128, 128], [1, 121]]),
        in_=AP(img.tensor, 13 * 38 + 13, [[1444, 128], [38, 11], [1, 11]]),
    )
    ps_a = pcorr.tile([128, 392], F32, tag="corr_a")
    ps_b = pcorr.tile([128, 392], F32, tag="corr_b")
    return dict(img=img, tmpl=tmpl, ps_a=ps_a, ps_b=ps_b)


def _corr_taps(nc, cx, t0, t1, P, C):
    """Accumulating diag matmuls for taps [t0, t1)."""
    img, tmpl = cx["img"], cx["tmpl"]
    ps_a, ps_b = cx["ps_a"], cx["ps_b"]
    for t in range(t0, t1):
        u, v = t // 11, t % 11
        dg = P["diagpool"].tile([128, 128], BF16)
        nc.vector.tensor_scalar_mul(dg[:, :], C["ident"][:, :], tmpl[:, t : t + 1])
        if t == 0:
            # full extent (zero-padded rows contribute 0) to reset PSUM
            y0, y1, x0, x1 = 0, 28, 0, 28
        else:
            # rows/cols whose img window is entirely zero padding are skipped
            y0, y1 = max(0, 5 - u), min(28, 33 - u)
            x0, x1 = max(0, 5 - v), min(28, 33 - v)
        ya0, ya1 = y0, min(14, y1)
        yb0, yb1 = max(14, y0), y1
        nc.tensor.matmul(
            AP(ps_a.tensor, ya0 * 28 + x0, [[392, 128], [28, ya1 - ya0], [1, x1 - x0]]),
            dg[:, :],
            AP(
                img.tensor,
                (ya0 + u) * 38 + (x0 + v),
                [[1444, 128], [38, ya1 - ya0], [1, x1 - x0]],
            ),
            start=(t == 0), stop=(t == 120), skip_group_check=True,
        )
        nc.tensor.matmul(
            AP(ps_b.tensor, (yb0 - 14) * 28 + x0, [[392, 128], [28, yb1 - yb0], [1, x1 - x0]]),
            dg[:, :],
            AP(
                img.tensor,
                (yb0 + u) * 38 + (x0 + v),
                [[1444, 128], [38, yb1 - yb0], [1, x1 - x0]],
            ),
            start=(t == 0), stop=(t == 120), skip_group_check=True,
        )


def _corr_end(nc, b, cx, P, C):
    corr = P["corrpool"].tile([128, 924], BF16)
    nc.vector.tensor_copy(out=corr[:, 0:392], in_=cx["ps_a"][:, :])
    nc.vector.tensor_copy(out=corr[:, 392:784], in_=cx["ps_b"][:, :])
    nc.gpsimd.memset(corr[:, 784:924], 0.0)
    corr_d = P["dscrpool"].tile([136, 924], BF16, tag="corr_d")
    nc.sync.dma_start(
        out=AP(corr_d.tensor, 0, [[924, 128], [1, 924]]),
        in_=corr[:, :],
    )
    # zero tail rows (s25g shifted reads overrun into them)
    nc.sync.dma_start(
        out=AP(corr_d.tensor, 128 * 924, [[924, 8], [1, 924]]),
        in_=C["zpad"][:, :],
    )
    return corr_d


def _conv_stage(nc, b, corr_d, out_d, P, C, filler=None, finish=None, pre=None):
    logitsb = P["lgbpool"].tile([16, 128], F32)
    # sub-level software pipeline, depth 2: A(s+2) is issued before B(s)
    # so PE always has conv1 work while dup DMAs for the next B transfer.
    # filler(k, 5) pads each B stage with the NEXT btile's corr taps;
    # finish() flushes that corr to DRAM during the tail B stages so the
    # next btile's first s25g prefetches cross the boundary early.
    dups = {}
    l3s = {}
    if pre is None:
        s25 = {0: _s25g_fetch(nc, 0, corr_d, P), 1: _s25g_fetch(nc, 1, corr_d, P)}
    else:
        s25 = pre
    nxt = None
    for s in range(6):
        if s + 2 < 4:
            s25[s + 2] = _s25g_fetch(nc, s + 2, corr_d, P)
        if filler is not None and s < 5:
            filler(2 * s, 10)
        if s < 4:
            dups[s] = _conv_a(nc, b, s, s25.pop(s), P, C)
        if filler is not None and s < 5:
            filler(2 * s + 1, 10)
        if s == 5 and finish is not None:
            nxt = finish()
        if s >= 2:
            _conv_b(nc, b, s - 2, dups.pop(s - 2), logitsb, l3s, P, C)
    _softmax_out(nc, b, logitsb, out_d, P, C)
    return nxt


def _s25g_fetch(nc, sub, corr_d, P):
    """im2col prefetch: s25g[20*dy+4*dx+g, c*924+j] =
    corr[32*sub+8*g+c, dy*28+dx + j]"""
    s25g = P["s25pool"].tile([100, 7392], BF16)
    for dy in range(5):
        nc.gpsimd.dma_start(
            out=s25g[20 * dy : 20 * dy + 20, :],
            in_=AP(
                corr_d.tensor,
                sub * 32 * 924 + dy * 28,
                [[1, 5], [7392, 4], [1, 7392]],
            ),
        )
    return s25g


def _conv_a(nc, b, sub, s25g, P, C):
    """conv1 -> maxpool (raw) -> bias+relu -> shifted dups."""
    w1x4_sb = C["w1x4_sb"]; b1x4_sb = C["b1x4_sb"]
    px_all = P["pxpool"].tile([128, 2304], BF16)
    for c in range(8):
        for h in range(2):
            ps1 = P["pc1"].tile([128, 288], F32, tag="ps1")
            nc.tensor.matmul(
                ps1[:, :],
                w1x4_sb[0:100, :],
                AP(
                    s25g.tensor,
                    c * 924 + h * 336,
                    [[7392, 100], [28, 12], [1, 24]],
                ),
                start=True, stop=True,
            )
            # maxpool x-pairs straight off PSUM (bias/relu commute with max)
            nc.vector.tensor_reduce(
                out=px_all[:, c * 288 + h * 144 : c * 288 + h * 144 + 144],
                in_=AP(ps1.tensor, 0, [[288, 128], [24, 12], [2, 12], [1, 2]]),
                axis=AXIS.X,
                op=ALU.max,
            )
    # maxpool y-pairs then one bias+relu -> pooled [128=(4co+g), (c,12,12)]
    py_raw = P["pyrpool"].tile([128, 1152], BF16)
    nc.vector.tensor_max(
        py_raw[:, :],
        AP(px_all.tensor, 0, [[2304, 128], [24, 96], [1, 12]]),
        AP(px_all.tensor, 12, [[2304, 128], [24, 96], [1, 12]]),
    )
    py_all = P["pypool"].tile([128, 1204], BF16)
    nc.scalar.activation(
        py_all[:, 0:1152], py_raw[:, :], ACTF.Relu, bias=b1x4_sb[:, 0:1]
    )
    nc.gpsimd.memset(py_all[:, 1152:1204], 0.0)
    # shifted dups (partition remap to [32ci, ...]):
    # dup_A row-group r bakes dx=r (shift r elems); dup2 bakes dx=4
    dup_A = P["dupApool"].tile([128, 4612], BF16)
    dup2 = P["dup2pool"].tile([32, 4608], BF16)
    for r in range(4):
        nc.sync.dma_start(
            out=AP(
                dup_A.tensor,
                32 * r * 4612 + (4 - r),
                [[4612, 32], [1152, 4], [1, 1152]],
            ),
            in_=AP(py_all.tensor, 0, [[1204, 128], [1, 1152]]),
        )
    nc.sync.dma_start(
        out=AP(dup2.tensor, 0, [[4608, 32], [1152, 4], [1, 1152]]),
        in_=AP(py_all.tensor, 4, [[1204, 128], [1, 1152]]),
    )
    return dup_A, dup2


def _conv_b(nc, b, sub, dup_pair, logitsb, l3s, P, C):
    """conv2 (dy via rhs offset, dx via dup row-groups) -> pool -> conv3 -> GAP."""
    dup_A, dup2 = dup_pair
    w2dx_sb = C["w2dx_sb"]; w2x4_sb = C["w2x4_sb"]; b2p_sb = C["b2p_sb"]
    w3n_sb = C["w3n_sb"]; b3q_sb = C["b3q_sb"]

    if sub % 2 == 0:
        l3s["t"] = P["l3pool"].tile([64, 1024], BF16, name="l3pair")
    l3 = l3s["t"]
    half = (sub % 2) * 512
    for cc in range(4):
        ps2 = P["pc2"].tile([64, 512], F32, tag="ps2")
        for dy in range(5):
            nc.tensor.matmul(
                ps2[:, :],
                w2dx_sb[:, 64 * dy : 64 * dy + 64],
                AP(
                    dup_A.tensor,
                    4 + cc * 1152 + dy * 12,
                    [[4612, 128], [144, 8], [12, 8], [1, 8]],
                ),
                start=(dy == 0), stop=False,
            )
        # taps (dy, dx=4): K=32 via dup2 (content pre-shifted by 4)
        for dy in range(5):
            nc.tensor.matmul(
                ps2[:, :],
                w2x4_sb[:, 64 * dy : 64 * dy + 64],
                AP(
                    dup2.tensor,
                    cc * 1152 + dy * 12,
                    [[4608, 32], [144, 8], [12, 8], [1, 8]],
                ),
                start=False, stop=(dy == 4),
                tile_position=(0, 0),
            )
        # bias+relu off PSUM on ACT, then maxpool 2x2 on DVE into l3
        o2 = P["o2pool"].tile([64, 512], BF16)
        nc.scalar.activation(o2[:, :], ps2[:, :], ACTF.Relu, bias=b2p_sb[:, 0:1])
        qx = P["qxpool"].tile([64, 256], BF16)
        nc.vector.tensor_reduce(
            out=qx[:, :],
            in_=AP(o2.tensor, 0, [[512, 64], [8, 64], [2, 4], [1, 2]]),
            axis=AXIS.X,
            op=ALU.max,
        )
        nc.vector.tensor_max(
            l3[:, half + cc * 128 : half + cc * 128 + 128],
            AP(qx.tensor, 0, [[256, 64], [32, 8], [8, 4], [1, 4]]),
            AP(qx.tensor, 4, [[256, 64], [32, 8], [8, 4], [1, 4]]),
        )

    if sub % 2 == 0:
        return
    # bias+relu once for the pair -> conv3 input [64ci, (64s,16pix)]
    # conv3: 9 taps, K=64, one PSUM region, both subs in one N=256 stream
    ps3 = P["pc3"].tile([16, 256], F32, tag="ps3")
    for t in range(9):
        dy, dx = t // 3, t % 3
        nc.tensor.matmul(
            ps3[0:10, :],
            w3n_sb[0:64, 10 * t : 10 * t + 10],
            AP(l3.tensor, dy * 4 + dx, [[1024, 64], [16, 64], [4, 2], [1, 2]]),
            start=(t == 0), stop=(t == 8),
        )
    # relu(0.25*x + 0.25*b3) then sum 4 pix = GAP of relu(x+b3)
    ga = P["smpool"].tile([16, 256], F32, tag="ga")
    nc.scalar.activation(
        ga[0:10, :], ps3[0:10, :], ACTF.Relu, bias=b3q_sb[0:10, 0:1], scale=0.25
    )
    nc.vector.tensor_reduce(
        out=logitsb[0:10, (sub - 1) * 32 : (sub + 1) * 32],
        in_=AP(ga.tensor, 0, [[256, 10], [4, 64], [1, 4]]),
        axis=AXIS.X,
        op=ALU.add,
    )


def _softmax_out(nc, b, logitsb, out_d, P, C):
    smpool = P["smpool"]
    psT = P["pc1"].tile([128, 16], F32, tag="ps1")
    nc.tensor.transpose(psT[:, 0:10], logitsb[0:10, :], C["ident10"][0:10, 0:10])
    mx = smpool.tile([128, 1], F32, tag="mx")
    nc.vector.reduce_max(mx[:, :], psT[:, 0:10], axis=AXIS.X)
    hs = smpool.tile([128, 16], F32, tag="hs")
    nc.vector.tensor_scalar(hs[:, 0:10], psT[:, 0:10], mx[:, 0:1], None, ALU.subtract)
    ex = smpool.tile([128, 16], F32, tag="ex")
    nc.scalar.activation(ex[:, 0:10], hs[:, 0:10], ACTF.Exp)
    sm = smpool.tile([128, 1], F32, tag="sm")
    nc.vector.reduce_sum(sm[:, :], ex[:, 0:10], axis=AXIS.X)
    lsm = smpool.tile([128, 1], F32, tag="lsm")
    nc.scalar.activation(lsm[:, :], sm[:, :], ACTF.Ln)
    outt = smpool.tile([128, 16], F32, tag="outt")
    nc.vector.tensor_scalar(outt[:, 0:10], hs[:, 0:10], lsm[:, 0:1], None, ALU.subtract)
    nc.sync.dma_start(
        out=AP(out_d, b * 1280, [[10, 128], [1, 10]]),
        in_=outt[:, 0:10],
    )


_CACHE = {}


def _get_nc(b_core):
    if b_core not in _CACHE:
        nc = bacc.Bacc(
            "TRN2",
            target_bir_lowering=False,
            debug=False,
            num_devices=N_CORES,
            num_swdge_queues=2,
        )
        _build(nc, b_core)
        nc.compile()
        _CACHE[b_core] = nc
    return _CACHE[b_core]


def _prep_inputs(inputs):
    import ml_dtypes

    bf16 = ml_dtypes.bfloat16
    w1 = np.asarray(inputs["w1"], dtype=np.float32).reshape(32, 25)
    w2 = np.asarray(inputs["w2"], dtype=np.float32).reshape(64, 32, 5, 5)
    w3 = np.asarray(inputs["w3"], dtype=np.float32).reshape(10, 64, 9)
    b1 = np.asarray(inputs["b1"], dtype=np.float32)
    b2 = np.asarray(inputs["b2"], dtype=np.float32)
    b3 = np.asarray(inputs["b3"], dtype=np.float32)

    # conv1: w1x4[4*t+g, 4*co+g] = w1[co, t]
    w1x4 = np.zeros((100, 128), dtype=np.float32)
    for t in range(25):
        for g in range(4):
            w1x4[4 * t + g, 4 * np.arange(32) + g] = w1[:, t]
    b1x4 = np.zeros((128, 1), dtype=np.float32)
    for co in range(32):
        for g in range(4):
            b1x4[4 * co + g, 0] = b1[co]

    # conv2: w2dx[32*r+ci, 64*dy+co] = w2[co, ci, dy, r] (r=dx 0..3)
    w2dx = np.zeros((128, 320), dtype=np.float32)
    for r in range(4):
        for dy in range(5):
            w2dx[32 * r : 32 * r + 32, 64 * dy : 64 * dy + 64] = w2[:, :, dy, r].T
    # w2x4[ci, 64*dy+co] = w2[co, ci, dy, 4]
    w2x4 = np.zeros((32, 320), dtype=np.float32)
    for dy in range(5):
        w2x4[:, 64 * dy : 64 * dy + 64] = w2[:, :, dy, 4].T

    # conv3: w3n[ci, 10*t+co] = w3[co, ci, t]
    w3n = np.zeros((64, 96), dtype=np.float32)
    for t in range(9):
        w3n[:, 10 * t : 10 * t + 10] = w3[:, :, t].T
    b3q = np.zeros((16, 1), dtype=np.float32)
    b3q[0:10, 0] = 0.25 * b3

    return dict(
        identp=np.eye(128, dtype=bf16),
        ident10p=np.eye(16, dtype=np.float32),
        w1x4=w1x4.astype(bf16),
        b1x4=b1x4,
        w2dx=w2dx.astype(bf16),
        w2x4=w2x4.astype(bf16),
        b2p=b2.reshape(64, 1),
        w3n=w3n.astype(bf16),
        b3q=b3q,
    )


def _run(inputs, b_core=B_CORE, trace=False):
    x = np.ascontiguousarray(np.asarray(inputs["x"], dtype=np.float32))
    consts = _prep_inputs(inputs)
    nc = _get_nc(b_core)
    in_maps = [
        {"x": x[i * b_core : (i + 1) * b_core], **consts} for i in range(N_CORES)
    ]
    res = run_bass_kernel_spmd(nc, in_maps, core_ids=list(range(N_CORES)), trace=trace)
    out = np.concatenate([res.results[i]["out"] for i in range(N_CORES)], axis=0)
    return out.astype(np.float32), res


def kernel(**inputs) -> np.ndarray:
    out, _ = _run(inputs)
    return out


# revision 48
# speedup vs baseline: 1.0663x; 1.0539x over previous
"""Trainium2 Bass kernel for nn_Net_76330158785143 (dense_cnn).

Pipeline per sample: per-sample 11x11 autocorrelation of channel 2 ->
conv5x5(1->32) relu -> maxpool2 -> conv5x5(32->64) relu -> maxpool2 ->
conv3x3(64->10) relu -> GAP -> log_softmax.

Sharding: pure data parallel, batch 8192 -> 1024 per core across 8 cores.

Layout notes (per 128-sample btile, 4 subs of 32 samples each):
- sample-in-sub index s = 8*g + c  (g: partition-group 0..3, c: chunk 0..7)
- corr: per-tap diag matmuls accumulating in PSUM, out columns split in two
  392-wide halves (PSUM bank cap), zero-padding rows/cols trimmed per tap
- conv1: 4 samples stacked on PE rows (K=4x25=100), out partition m=4*co+g
- conv2: dx 0..3 baked into dup_A row-groups (K=128, one matmul per dy);
  dx=4 taps as K=32 matmuls from dup2 (content pre-shifted by 4)
- conv3: all 9 taps accumulate into one PSUM region (K=64), two subs per
  N=256 stream
- schedule: corr(b+1) tap chunks interleaved into conv(b)'s sub-pipeline
  slots (depth-2 A/B software pipeline, s25g im2col prefetched 2 subs
  ahead and across the btile boundary via finish())
"""

import sys

sys.path.insert(0, "/opt/trn_rl_repo")

import numpy as np

import concourse.bacc as bacc
import concourse.mybir as mybir
from concourse.ap import AP
from concourse.tile import TileContext
from concourse.bass_utils import run_bass_kernel_spmd

F32 = mybir.dt.float32
BF16 = mybir.dt.bfloat16
ALU = mybir.AluOpType
ACTF = mybir.ActivationFunctionType
AXIS = mybir.AxisListType

N_CORES = 8
B_FULL = 8192
B_CORE = B_FULL // N_CORES


def _build(nc, b_core):
    n_bt = b_core // 128

    x_d = nc.dram_tensor("x", [b_core, 3, 28, 28], F32, kind="ExternalInput")
    identp_d = nc.dram_tensor("identp", [128, 128], BF16, kind="ExternalInput")
    ident10p_d = nc.dram_tensor("ident10p", [16, 16], F32, kind="ExternalInput")
    w1x4_d = nc.dram_tensor("w1x4", [100, 128], BF16, kind="ExternalInput")
    b1x4_d = nc.dram_tensor("b1x4", [128, 1], F32, kind="ExternalInput")
    w2dx_d = nc.dram_tensor("w2dx", [128, 320], BF16, kind="ExternalInput")
    w2y4_d = nc.dram_tensor("w2y4", [64, 128], BF16, kind="ExternalInput")
    w2c_d = nc.dram_tensor("w2c", [32, 64], BF16, kind="ExternalInput")
    b2p_d = nc.dram_tensor("b2p", [64, 1], F32, kind="ExternalInput")
    w3n_d = nc.dram_tensor("w3n", [64, 96], BF16, kind="ExternalInput")
    b3q_d = nc.dram_tensor("b3q", [16, 1], F32, kind="ExternalInput")
    out_d = nc.dram_tensor("out", [b_core, 10], F32, kind="ExternalOutput")

    with TileContext(nc) as tc:
        cpool_cm = tc.tile_pool(name="const", bufs=1)
        cpool = cpool_cm.__enter__()

        def _load_const(name, dram, shape, dtype):
            t = cpool.tile(shape, dtype, name=name + "_sb")
            f = int(np.prod(shape[1:]))
            nc.sync.dma_start(
                out=AP(t.tensor, 0, [[f, shape[0]], [1, f]]),
                in_=AP(dram, 0, [[f, shape[0]], [1, f]]),
            )
            return t

        ident = _load_const("ident", identp_d, [128, 128], BF16)
        ident10 = _load_const("ident10", ident10p_d, [16, 16], F32)
        w1x4_sb = _load_const("w1x4", w1x4_d, [100, 128], BF16)
        b1x4_sb = _load_const("b1x4", b1x4_d, [128, 1], F32)
        w2dx_sb = _load_const("w2dx", w2dx_d, [128, 320], BF16)
        w2y4_sb = _load_const("w2y4", w2y4_d, [64, 128], BF16)
        w2c_sb = _load_const("w2c", w2c_d, [32, 64], BF16)
        b2p_sb = _load_const("b2p", b2p_d, [64, 1], F32)
        w3n_sb = _load_const("w3n", w3n_d, [64, 96], BF16)
        b3q_sb = _load_const("b3q", b3q_d, [16, 1], F32)
        # zero pad rows for corr_d tail (s25g shifted reads run past row 128)
        zpad = cpool.tile([8, 924], BF16, name="zpad_sb")
        nc.vector.memset(zpad[:, :], 0.0)

        from contextlib import ExitStack

        with ExitStack() as stack:
            ent = stack.enter_context
            imgpool = ent(tc.tile_pool(name="img", bufs=2))
            tmplpool = ent(tc.tile_pool(name="tmpl", bufs=2))
            diagpool = ent(tc.tile_pool(name="diag", bufs=8))
            corrpool = ent(tc.tile_pool(name="corr", bufs=2))
            s25pool = ent(tc.tile_pool(name="s25", bufs=6))
            pxpool = ent(tc.tile_pool(name="px", bufs=3))
            pyrpool = ent(tc.tile_pool(name="pyr", bufs=2))
            pypool = ent(tc.tile_pool(name="py", bufs=3))
            dupApool = ent(tc.tile_pool(name="dupA", bufs=3))
            dup2pool = ent(tc.tile_pool(name="dup2", bufs=3))
            o2pool = ent(tc.tile_pool(name="o2", bufs=4))
            qxpool = ent(tc.tile_pool(name="qx", bufs=4))
            l3pool = ent(tc.tile_pool(name="l3", bufs=2))
            smpool = ent(tc.tile_pool(name="sm", bufs=4))
            lgbpool = ent(tc.tile_pool(name="lgb", bufs=2))
            dscrpool = ent(tc.tile_pool(name="dscr", bufs=2, space="DRAM"))
            pcorr = ent(tc.tile_pool(name="pcorr", bufs=1, space="PSUM"))
            pc1 = ent(tc.tile_pool(name="pc1", bufs=3, space="PSUM"))
            pc2 = ent(tc.tile_pool(name="pc2", bufs=2, space="PSUM"))
            pc3 = ent(tc.tile_pool(name="pc3", bufs=1, space="PSUM"))
            pools = dict(
                imgpool=imgpool, tmplpool=tmplpool, diagpool=diagpool,
                corrpool=corrpool, s25pool=s25pool,
                pxpool=pxpool, pyrpool=pyrpool, pypool=pypool,
                dupApool=dupApool, dup2pool=dup2pool, o2pool=o2pool,
                qxpool=qxpool, l3pool=l3pool, smpool=smpool,
                lgbpool=lgbpool, dscrpool=dscrpool, pcorr=pcorr,
                pc1=pc1, pc2=pc2, pc3=pc3,
            )
            consts = dict(
                ident=ident, ident10=ident10, w1x4_sb=w1x4_sb,
                b1x4_sb=b1x4_sb, w2dx_sb=w2dx_sb, w2y4_sb=w2y4_sb,
                w2c_sb=w2c_sb,
                b2p_sb=b2p_sb, w3n_sb=w3n_sb,
                b3q_sb=b3q_sb, zpad=zpad,
            )
            # software pipeline: corr(b) tap chunks are interleaved into
            # conv(b-1)'s sub-pipeline slots so every dup-DMA latency
            # window has PE work to chew on.
            state = {}
            ctxs = {}
            pre = None
            for b in range(n_bt + 1):
                if b < n_bt:
                    ctxs[b] = _corr_begin(nc, b, x_d, pools, consts)
                if b >= 1:
                    cx = ctxs.get(b)
                    # btile 1 has no cross-boundary s25g prefetch; put some
                    # of corr(1)'s taps in front of conv(0) to cover the
                    # corr_d(0) -> s25g(0) handoff latency
                    ts = 0
                    if b == 1 and cx is not None:
                        _corr_taps(nc, cx, 0, 25, pools, consts)
                        ts = 25

                    def filler(k, nslots, cx=cx, ts=ts):
                        if cx is None:
                            return
                        t0 = ts + ((121 - ts) * k) // nslots
                        t1 = ts + ((121 - ts) * (k + 1)) // nslots
                        if t1 > t0:
                            _corr_taps(nc, cx, t0, t1, pools, consts)

                    def finish(b=b, cx=cx):
                        if cx is None:
                            return None
                        cd = _corr_end(nc, b, cx, pools, consts)
                        state[b] = cd
                        ctxs.pop(b)
                        return {
                            0: _s25g_fetch(nc, 0, cd, pools),
                            1: _s25g_fetch(nc, 1, cd, pools),
                        }

                    pre = _conv_stage(
                        nc, b - 1, state.pop(b - 1), out_d, pools, consts,
                        filler, finish, pre,
                    )
                else:
                    _corr_taps(nc, ctxs[0], 0, 121, pools, consts)
                    state[0] = _corr_end(nc, 0, ctxs.pop(0), pools, consts)

        cpool_cm.__exit__(None, None, None)
    return nc


def _corr_begin(nc, b, x_d, P, C):
    imgpool = P["imgpool"]; tmplpool = P["tmplpool"]; pcorr = P["pcorr"]
    # channel 2 into zero-padded 38x38, cast bf16
    img = imgpool.tile([128, 38 * 38], BF16)
    nc.gpsimd.memset(img[:, :], 0.0)
    nc.gpsimd.dma_start(
        out=AP(img.tensor, 5 * 38 + 5, [[1444, 128], [38, 28], [1, 28]]),
        in_=AP(x_d, b * 128 * 2352 + 2 * 784, [[2352, 128], [1, 784]]),
    )
    # template = center 11x11 crop
    tmpl = tmplpool.tile([128, 128], F32)
    nc.vector.tensor_copy(
        out=AP(tmpl.tensor, 0, [[128, 128], [1, 121]]),
        in_=AP(img.tensor, 13 * 38 + 13, [[1444, 128], [38, 11], [1, 11]]),
    )
    ps_a = pcorr.tile([128, 392], F32, tag="corr_a")
    ps_b = pcorr.tile([128, 392], F32, tag="corr_b")
    return dict(img=img, tmpl=tmpl, ps_a=ps_a, ps_b=ps_b)


def _corr_taps(nc, cx, t0, t1, P, C):
    """Accumulating diag matmuls for taps [t0, t1)."""
    img, tmpl = cx["img"], cx["tmpl"]
    ps_a, ps_b = cx["ps_a"], cx["ps_b"]
    for t in range(t0, t1):
        u, v = t // 11, t % 11
        dg = P["diagpool"].tile([128, 128], BF16)
        nc.vector.tensor_scalar_mul(dg[:, :], C["ident"][:, :], tmpl[:, t : t + 1])
        if t == 0:
            # full extent (zero-padded rows contribute 0) to reset PSUM
            y0, y1, x0, x1 = 0, 28, 0, 28
        else:
            # rows/cols whose img window is entirely zero padding are skipped
            y0, y1 = max(0, 5 - u), min(28, 33 - u)
            x0, x1 = max(0, 5 - v), min(28, 33 - v)
        ya0, ya1 = y0, min(14, y1)
        yb0, yb1 = max(14, y0), y1
        nc.tensor.matmul(
            AP(ps_a.tensor, ya0 * 28 + x0, [[392, 128], [28, ya1 - ya0], [1, x1 - x0]]),
            dg[:, :],
            AP(
                img.tensor,
                (ya0 + u) * 38 + (x0 + v),
                [[1444, 128], [38, ya1 - ya0], [1, x1 - x0]],
            ),
            start=(t == 0), stop=(t == 120), skip_group_check=True,
        )
        nc.tensor.matmul(
            AP(ps_b.tensor, (yb0 - 14) * 28 + x0, [[392, 128], [28, yb1 - yb0], [1, x1 - x0]]),
            dg[:, :],
            AP(
                img.tensor,
                (yb0 + u) * 38 + (x0 + v),
                [[1444, 128], [38, yb1 - yb0], [1, x1 - x0]],
            ),
            start=(t == 0), stop=(t == 120), skip_group_check=True,
        )


def _corr_end(nc, b, cx, P, C):
    corr = P["corrpool"].tile([128, 924], BF16)
    nc.vector.tensor_copy(out=corr[:, 0:392], in_=cx["ps_a"][:, :])
    nc.vector.tensor_copy(out=corr[:, 392:784], in_=cx["ps_b"][:, :])
    nc.gpsimd.memset(corr[:, 784:924], 0.0)
    corr_d = P["dscrpool"].tile([136, 924], BF16, tag="corr_d")
    nc.sync.dma_start(
        out=AP(corr_d.tensor, 0, [[924, 128], [1, 924]]),
        in_=corr[:, :],
    )
    # zero tail rows (s25g shifted reads overrun into them)
    nc.sync.dma_start(
        out=AP(corr_d.tensor, 128 * 924, [[924, 8], [1, 924]]),
        in_=C["zpad"][:, :],
    )
    return corr_d


def _conv_stage(nc, b, corr_d, out_d, P, C, filler=None, finish=None, pre=None):
    logitsb = P["lgbpool"].tile([16, 128], F32)
    # sub-level software pipeline, depth 2: A(s+2) is issued before B(s)
    # so PE always has conv1 work while dup DMAs for the next B transfer.
    # filler(k, 5) pads each B stage with the NEXT btile's corr taps;
    # finish() flushes that corr to DRAM during the tail B stages so the
    # next btile's first s25g prefetches cross the boundary early.
    dups = {}
    l3s = {}
    if pre is None:
        s25 = {0: _s25g_fetch(nc, 0, corr_d, P), 1: _s25g_fetch(nc, 1, corr_d, P)}
    else:
        s25 = pre
    nxt = None
    for s in range(6):
        if s + 2 < 4:
            s25[s + 2] = _s25g_fetch(nc, s + 2, corr_d, P)
        if filler is not None and s < 5:
            filler(2 * s, 10)
        if s < 4:
            dups[s] = _conv_a(nc, b, s, s25.pop(s), P, C)
        if filler is not None and s < 5:
            filler(2 * s + 1, 10)
        if s == 5 and finish is not None:
            nxt = finish()
        if s >= 2:
            _conv_b(nc, b, s - 2, dups.pop(s - 2), logitsb, l3s, P, C)
    _softmax_out(nc, b, logitsb, out_d, P, C)
    return nxt


def _s25g_fetch(nc, sub, corr_d, P):
    """im2col prefetch: s25g[20*dy+4*dx+g, c*924+j] =
    corr[32*sub+8*g+c, dy*28+dx + j]"""
    s25g = P["s25pool"].tile([100, 7392], BF16)
    for dy in range(5):
        nc.gpsimd.dma_start(
            out=s25g[20 * dy : 20 * dy + 20, :],
            in_=AP(
                corr_d.tensor,
                sub * 32 * 924 + dy * 28,
                [[1, 5], [7392, 4], [1, 7392]],
            ),
        )
    return s25g


def _conv_a(nc, b, sub, s25g, P, C):
    """conv1 -> maxpool (raw) -> bias+relu -> shifted dups."""
    w1x4_sb = C["w1x4_sb"]; b1x4_sb = C["b1x4_sb"]
    px_all = P["pxpool"].tile([128, 2304], BF16)
    for c in range(8):
        for h in range(2):
            ps1 = P["pc1"].tile([128, 288], F32, tag="ps1")
            nc.tensor.matmul(
                ps1[:, :],
                w1x4_sb[0:100, :],
                AP(
                    s25g.tensor,
                    c * 924 + h * 336,
                    [[7392, 100], [28, 12], [1, 24]],
                ),
                start=True, stop=True,
            )
            # maxpool x-pairs straight off PSUM (bias/relu commute with max)
            nc.vector.tensor_reduce(
                out=px_all[:, c * 288 + h * 144 : c * 288 + h * 144 + 144],
                in_=AP(ps1.tensor, 0, [[288, 128], [24, 12], [2, 12], [1, 2]]),
                axis=AXIS.X,
                op=ALU.max,
            )
    # maxpool y-pairs then one bias+relu -> pooled [128=(4co+g), (c,12,12)]
    py_raw = P["pyrpool"].tile([128, 1152], BF16)
    nc.vector.tensor_max(
        py_raw[:, :],
        AP(px_all.tensor, 0, [[2304, 128], [24, 96], [1, 12]]),
        AP(px_all.tensor, 12, [[2304, 128], [24, 96], [1, 12]]),
    )
    py_all = P["pypool"].tile([128, 1204], BF16)
    nc.scalar.activation(
        py_all[:, 0:1152], py_raw[:, :], ACTF.Relu, bias=b1x4_sb[:, 0:1]
    )
    nc.gpsimd.memset(py_all[:, 1152:1204], 0.0)
    # shifted dups (partition remap to [32ci, ...]):
    # dup_A row-group r bakes dx=r (shift r elems); dup2 bakes dx=4
    dup_A = P["dupApool"].tile([128, 4612], BF16)
    dup_B2h = P["dup2pool"].tile([64, 4624], BF16, name="dupb2h")
    for r in range(4):
        nc.sync.dma_start(
            out=AP(
                dup_A.tensor,
                32 * r * 4612 + (4 - r),
                [[4612, 32], [1152, 4], [1, 1152]],
            ),
            in_=AP(py_all.tensor, 0, [[1204, 128], [1, 1152]]),
        )
    # dup_B2h row-group r bakes shift 4+12r (taps (dy,4): base 12*dy picks
    # the dy pair; r selects dy parity)
    for r in range(2):
        nc.sync.dma_start(
            out=AP(
                dup_B2h.tensor,
                32 * r * 4624 + (12 - 12 * r),
                [[4624, 32], [1152, 4], [1, 1152]],
            ),
            in_=AP(py_all.tensor, 0, [[1204, 128], [1, 1152]]),
        )
    return dup_A, dup_B2h


def _conv_b(nc, b, sub, dup_pair, logitsb, l3s, P, C):
    """conv2 (dy via rhs offset, dx via dup row-groups) -> pool -> conv3 -> GAP."""
    dup_A, dup_B2h = dup_pair
    w2dx_sb = C["w2dx_sb"]; w2y4_sb = C["w2y4_sb"]; w2c_sb = C["w2c_sb"]
    b2p_sb = C["b2p_sb"]
    w3n_sb = C["w3n_sb"]; b3q_sb = C["b3q_sb"]

    if sub % 2 == 0:
        l3s["t"] = P["l3pool"].tile([64, 1024], BF16, name="l3pair")
    l3 = l3s["t"]
    half = (sub % 2) * 512
    for cc in range(4):
        ps2 = P["pc2"].tile([64, 512], F32, tag="ps2")
        for dy in range(5):
            nc.tensor.matmul(
                ps2[:, :],
                w2dx_sb[:, 64 * dy : 64 * dy + 64],
                AP(
                    dup_A.tensor,
                    4 + cc * 1152 + dy * 12,
                    [[4612, 128], [144, 8], [12, 8], [1, 8]],
                ),
                start=(dy == 0), stop=False,
            )
        # taps (dy,4) in dy-pairs: K=64 over dup_B2h row-groups
        for j in range(2):
            nc.tensor.matmul(
                ps2[:, :],
                w2y4_sb[:, 64 * j : 64 * j + 64],
                AP(
                    dup_B2h.tensor,
                    16 + cc * 1152 + 24 * j,
                    [[4624, 64], [144, 8], [12, 8], [1, 8]],
                ),
                start=False, stop=False,
                tile_position=(0, 0),
            )
        # tap (4,4): K=32 from row-group 0 (shift 4) at base 48
        nc.tensor.matmul(
            ps2[:, :],
            w2c_sb[:, :],
            AP(
                dup_B2h.tensor,
                16 + cc * 1152 + 48,
                [[4624, 32], [144, 8], [12, 8], [1, 8]],
            ),
            start=False, stop=True,
            tile_position=(0, 0),
        )
        # bias+relu off PSUM on ACT, then maxpool 2x2 on DVE into l3
        o2 = P["o2pool"].tile([64, 512], BF16)
        nc.scalar.activation(o2[:, :], ps2[:, :], ACTF.Relu, bias=b2p_sb[:, 0:1])
        qx = P["qxpool"].tile([64, 256], BF16)
        nc.vector.tensor_reduce(
            out=qx[:, :],
            in_=AP(o2.tensor, 0, [[512, 64], [8, 64], [2, 4], [1, 2]]),
            axis=AXIS.X,
            op=ALU.max,
        )
        nc.vector.tensor_max(
            l3[:, half + cc * 128 : half + cc * 128 + 128],
            AP(qx.tensor, 0, [[256, 64], [32, 8], [8, 4], [1, 4]]),
            AP(qx.tensor, 4, [[256, 64], [32, 8], [8, 4], [1, 4]]),
        )

    if sub % 2 == 0:
        return
    # bias+relu once for the pair -> conv3 input [64ci, (64s,16pix)]
    # conv3: 9 taps, K=64, one PSUM region, both subs in one N=256 stream
    ps3 = P["pc3"].tile([16, 256], F32, tag="ps3")
    for t in range(9):
        dy, dx = t // 3, t % 3
        nc.tensor.matmul(
            ps3[0:10, :],
            w3n_sb[0:64, 10 * t : 10 * t + 10],
            AP(l3.tensor, dy * 4 + dx, [[1024, 64], [16, 64], [4, 2], [1, 2]]),
            start=(t == 0), stop=(t == 8),
        )
    # relu(0.25*x + 0.25*b3) then sum 4 pix = GAP of relu(x+b3)
    ga = P["smpool"].tile([16, 256], F32, tag="ga")
    nc.scalar.activation(
        ga[0:10, :], ps3[0:10, :], ACTF.Relu, bias=b3q_sb[0:10, 0:1], scale=0.25
    )
    nc.vector.tensor_reduce(
        out=logitsb[0:10, (sub - 1) * 32 : (sub + 1) * 32],
        in_=AP(ga.tensor, 0, [[256, 10], [4, 64], [1, 4]]),
        axis=AXIS.X,
        op=ALU.add,
    )


def _softmax_out(nc, b, logitsb, out_d, P, C):
    smpool = P["smpool"]
    psT = P["pc1"].tile([128, 16], F32, tag="ps1")
    nc.tensor.transpose(psT[:, 0:10], logitsb[0:10, :], C["ident10"][0:10, 0:10])
    mx = smpool.tile([128, 1], F32, tag="mx")
    nc.vector.reduce_max(mx[:, :], psT[:, 0:10], axis=AXIS.X)
    hs = smpool.tile([128, 16], F32, tag="hs")
    nc.vector.tensor_scalar(hs[:, 0:10], psT[:, 0:10], mx[:, 0:1], None, ALU.subtract)
    ex = smpool.tile([128, 16], F32, tag="ex")
    nc.scalar.activation(ex[:, 0:10], hs[:, 0:10], ACTF.Exp)
    sm = smpool.tile([128, 1], F32, tag="sm")
    nc.vector.reduce_sum(sm[:, :], ex[:, 0:10], axis=AXIS.X)
    lsm = smpool.tile([128, 1], F32, tag="lsm")
    nc.scalar.activation(lsm[:, :], sm[:, :], ACTF.Ln)
    outt = smpool.tile([128, 16], F32, tag="outt")
    nc.vector.tensor_scalar(outt[:, 0:10], hs[:, 0:10], lsm[:, 0:1], None, ALU.subtract)
    nc.sync.dma_start(
        out=AP(out_d, b * 1280, [[10, 128], [1, 10]]),
        in_=outt[:, 0:10],
    )


_CACHE = {}


def _get_nc(b_core):
    if b_core not in _CACHE:
        nc = bacc.Bacc(
            "TRN2",
            target_bir_lowering=False,
            debug=False,
            num_devices=N_CORES,
            num_swdge_queues=2,
        )
        _build(nc, b_core)
        nc.compile()
        _CACHE[b_core] = nc
    return _CACHE[b_core]


def _prep_inputs(inputs):
    import ml_dtypes

    bf16 = ml_dtypes.bfloat16
    w1 = np.asarray(inputs["w1"], dtype=np.float32).reshape(32, 25)
    w2 = np.asarray(inputs["w2"], dtype=np.float32).reshape(64, 32, 5, 5)
    w3 = np.asarray(inputs["w3"], dtype=np.float32).reshape(10, 64, 9)
    b1 = np.asarray(inputs["b1"], dtype=np.float32)
    b2 = np.asarray(inputs["b2"], dtype=np.float32)
    b3 = np.asarray(inputs["b3"], dtype=np.float32)

    # conv1: w1x4[4*t+g, 4*co+g] = w1[co, t]
    w1x4 = np.zeros((100, 128), dtype=np.float32)
    for t in range(25):
        for g in range(4):
            w1x4[4 * t + g, 4 * np.arange(32) + g] = w1[:, t]
    b1x4 = np.zeros((128, 1), dtype=np.float32)
    for co in range(32):
        for g in range(4):
            b1x4[4 * co + g, 0] = b1[co]

    # conv2: w2dx[32*r+ci, 64*dy+co] = w2[co, ci, dy, r] (r=dx 0..3)
    w2dx = np.zeros((128, 320), dtype=np.float32)
    for r in range(4):
        for dy in range(5):
            w2dx[32 * r : 32 * r + 32, 64 * dy : 64 * dy + 64] = w2[:, :, dy, r].T
    # w2y4[32*r+ci, 64*j+co] = w2[co, ci, 2*j+r, 4]  (dy pairs)
    w2y4 = np.zeros((64, 128), dtype=np.float32)
    for j in range(2):
        for r in range(2):
            w2y4[32 * r : 32 * r + 32, 64 * j : 64 * j + 64] = w2[:, :, 2 * j + r, 4].T
    w2c = np.ascontiguousarray(w2[:, :, 4, 4].T)

    # conv3: w3n[ci, 10*t+co] = w3[co, ci, t]
    w3n = np.zeros((64, 96), dtype=np.float32)
    for t in range(9):
        w3n[:, 10 * t : 10 * t + 10] = w3[:, :, t].T
    b3q = np.zeros((16, 1), dtype=np.float32)
    b3q[0:10, 0] = 0.25 * b3

    return dict(
        identp=np.eye(128, dtype=bf16),
        ident10p=np.eye(16, dtype=np.float32),
        w1x4=w1x4.astype(bf16),
        b1x4=b1x4,
        w2dx=w2dx.astype(bf16),
        w2y4=w2y4.astype(bf16),
        w2c=w2c.astype(bf16),
        b2p=b2.reshape(64, 1),
        w3n=w3n.astype(bf16),
        b3q=b3q,
    )


def _run(inputs, b_core=B_CORE, trace=False):
    x = np.ascontiguousarray(np.asarray(inputs["x"], dtype=np.float32))
    consts = _prep_inputs(inputs)
    nc = _get_nc(b_core)
    in_maps = [
        {"x": x[i * b_core : (i + 1) * b_core], **consts} for i in range(N_CORES)
    ]
    res = run_bass_kernel_spmd(nc, in_maps, core_ids=list(range(N_CORES)), trace=trace)
    out = np.concatenate([res.results[i]["out"] for i in range(N_CORES)], axis=0)
    return out.astype(np.float32), res


def kernel(**inputs) -> np.ndarray:
    out, _ = _run(inputs)
    return out
